# revision 1
# baseline (speedup 1.0000x reference)
"""Trainium2 Bass kernel for nn_CrossAttention (B=32, S=512, D=768).

Reference computation per batch b:
    x1w    = x1[b] @ Wc                      # [S, D]
    x2r    = reshape(x2[b], (D, S))          # flat reinterpret, NOT transpose
    scores = x1w @ x2r                       # [S, S]
    A      = scores + (x1@W1)[:, None] + (x2@W2)[None, :]
    a1     = softmax(A, axis=0) ; a2 = softmax(A, axis=1)
    f_x1   = a1 @ x2 ;  f_x2 = a2.T @ x1     # each [S, D]

Sharding: data-parallel over batch, 4 batches per core on 8 cores, weights
replicated, no collectives.

Per-core dataflow (fp32 bits everywhere; matmuls run in float32r, which
streams 1 row/cycle at free-dim >= 256 vs 4 cycles/row for fp32):
    x1T  [d, s]  <- PE-transpose(x1)      (both mm1 operands contract over d)
    x1wT [e, s]  <- psum += Wc[dt, e_cols].T @ x1T[dt]        (mm1, 36 MM)
    scores[s, t] <- psum += x1wT[et, s_cols].T @ x2r[et]      (mm2, 24 MM)
                    + ones  (x)  b2row     K=1 bias fold-in   (4 MM)
    AT[t, s]     <- PE-transpose(A) accumulation + ones (x) b1row fold-in
                    (bias1 drops out of softmax over t; bias2 over s)
    a2 = softmax_free(A); a1T = softmax_free(AT)   (exp on ACT w/ accum)
    f_x1[s,:] = sum_tt a1T[tt, s_cols].T @ x2[tt]   (x2 natural layout)
    f_x2[t,:] = sum_st a2[st, t_cols].T @ x1[st]    (x1 natural layout)

Bias rows: b1row = W1col.T @ x1T on PE; b2row via gpsimd mul + DVE reduce
to columns, then four tiny [128,1] PE transposes.

Emission is software-pipelined across batches: the next batch's
transposes+mm1 are emitted between this batch's AT-softmax and f-matmuls,
so the PE never idles waiting on the softmax chain.
"""

import os
from contextlib import ExitStack

import numpy as np

import concourse.bacc as bacc
import concourse.mybir as mybir
import concourse.tile as tile
from concourse.bass_utils import run_bass_kernel_spmd
from concourse.masks import make_identity

B, S, D = 32, 512, 768
N_CORES = 8
BPC = B // N_CORES          # batches per core
P = 128                     # partitions
ST = S // P                 # 4 s/t tiles
DT = D // P                 # 6 d/e tiles
FC = 384                    # free-dim chunk for the f matmuls
FP32 = mybir.dt.float32
FP32R = mybir.dt.float32r
AX = mybir.AxisListType.X
EXP = mybir.ActivationFunctionType.Exp

MM_FAST = os.environ.get("XATTN_MM_FAST", "1") == "1"
PIPE = os.environ.get("XATTN_PIPE", "1") == "1"


def _mm(ap):
    """View for fp32r-matmul production/consumption."""
    return ap.bitcast(FP32R) if MM_FAST else ap


def _fp(ap):
    """fp32 view (vector/scalar-engine consumption of fp32r tiles)."""
    return ap.bitcast(FP32) if MM_FAST else ap


def build_kernel(repeat=1):
    nc = bacc.Bacc(None, target_bir_lowering=False)
    DT_IN = FP32R if MM_FAST else FP32
    x1 = nc.dram_tensor("x1", [BPC, S, D], DT_IN, kind="ExternalInput")
    x2 = nc.dram_tensor("x2", [BPC, S, D], DT_IN, kind="ExternalInput")
    Wc = nc.dram_tensor("Wc", [D, D], DT_IN, kind="ExternalInput")
    W1 = nc.dram_tensor("W1", [D], DT_IN, kind="ExternalInput")
    W2 = nc.dram_tensor("W2", [D], FP32, kind="ExternalInput")
    f1 = nc.dram_tensor("f1", [BPC, S, D], FP32, kind="ExternalOutput")
    f2 = nc.dram_tensor("f2", [BPC, S, D], FP32, kind="ExternalOutput")

    with ExitStack() as ctx:
        tc = ctx.enter_context(tile.TileContext(nc))
        consts = ctx.enter_context(tc.tile_pool(name="consts", bufs=1))
        pool_x = ctx.enter_context(tc.tile_pool(name="pool_x", bufs=int(os.environ.get("XATTN_XBUFS", "2"))))
        pool_big = ctx.enter_context(tc.tile_pool(name="pool_big", bufs=int(os.environ.get("XATTN_BIGBUFS", "1"))))
        pool_a = ctx.enter_context(tc.tile_pool(name="pool_a", bufs=1))
        pool_f = ctx.enter_context(tc.tile_pool(name="pool_f", bufs=int(os.environ.get("XATTN_FBUFS", "3"))))
        pool_sm = ctx.enter_context(tc.tile_pool(name="pool_sm", bufs=int(os.environ.get("XATTN_SMBUFS", "4"))))
        pool_scr = ctx.enter_context(tc.tile_pool(name="pool_scr", bufs=2))
        ps_tr = ctx.enter_context(tc.tile_pool(name="ps_tr", bufs=int(os.environ.get("XATTN_PSTR", "2")), space="PSUM"))
        ps_mm = ctx.enter_context(tc.tile_pool(name="ps_mm", bufs=int(os.environ.get("XATTN_PSMM", "3")), space="PSUM"))
        ps_f = ctx.enter_context(tc.tile_pool(name="ps_f", bufs=int(os.environ.get("XATTN_PSF", "2")), space="PSUM"))
        ps_sm = ctx.enter_context(tc.tile_pool(name="ps_sm", bufs=1, space="PSUM"))

        # batch-0 x1 tiles load first: the transposes (first PE work) need
        # them before anything else
        x1_sb_0 = pool_x.tile([P, ST, D], DT_IN, tag="x1")
        x1_view0 = x1[0].rearrange("(st p) d -> p st d", p=P)
        for st_ in range(ST):
            nc.sync.dma_start(out=x1_sb_0[:, st_, :], in_=x1_view0[:, st_, :])

        x2_sb_0 = pool_x.tile([P, ST, D], DT_IN, tag="x2")
        x2_view0 = x2[0].rearrange("(st p) d -> p st d", p=P)
        for st_ in range(ST):
            nc.sync.dma_start(out=x2_sb_0[:, st_, :], in_=x2_view0[:, st_, :])

        # ---- constants ----
        # identity first: it shares the Pool engine with the W2 broadcast,
        # and the first transposes need it
        identity_f = consts.tile([P, P], FP32)
        make_identity(nc, identity_f[:])
        identity = consts.tile([P, P], DT_IN)
        nc.vector.tensor_copy(identity[:], identity_f[:])
        ones_f = consts.tile([1, P], FP32)
        nc.vector.memset(ones_f[:], 1.0)
        ones_col = consts.tile([1, P], DT_IN)  # lhsT for K=1 bias-fold matmuls
        nc.vector.tensor_copy(ones_col[:], ones_f[:])
        Wc_sb = consts.tile([P, DT, D], DT_IN)  # [p, dt, e];  d = dt*128 + p
        Wc_view = Wc[:].rearrange("(dt p) e -> p dt e", p=P)
        for dt_ in range(DT):
            nc.sync.dma_start(out=Wc_sb[:, dt_, :], in_=Wc_view[:, dt_, :])
        W1col = consts.tile([P, DT], DT_IN)    # [p, dt]: W1[dt*128+p]
        nc.sync.dma_start(out=W1col[:], in_=W1[:].rearrange("(dt p) -> p dt", p=P))
        W2_rep = consts.tile([P, D], FP32)
        nc.gpsimd.dma_start(out=W2_rep[:], in_=W2[:].partition_broadcast(P))

        state = {}

        def emit_loads(i, b):
            if i == 0:
                x1_sb = x1_sb_0
                x2_sb = x2_sb_0
            else:
                x1_sb = pool_x.tile([P, ST, D], DT_IN, tag="x1")
                x1_view = x1[b].rearrange("(st p) d -> p st d", p=P)
                for st_ in range(ST):
                    nc.sync.dma_start(out=x1_sb[:, st_, :], in_=x1_view[:, st_, :])
                x2_sb = pool_x.tile([P, ST, D], DT_IN, tag="x2")
                x2_view = x2[b].rearrange("(st p) d -> p st d", p=P)
                for st_ in range(ST):
                    nc.sync.dma_start(out=x2_sb[:, st_, :], in_=x2_view[:, st_, :])
            # reshape(x2[b], [D, S]) is a flat reinterpret -> contiguous rows
            x2r_sb = pool_x.tile([P, DT, S], DT_IN, tag="x2r")
            x2r_view = (x2[b].rearrange("s d -> (s d)")
                        .rearrange("(et p t) -> p et t", p=P, t=S))
            for et in range(DT):
                nc.sync.dma_start(out=x2r_sb[:, et, :], in_=x2r_view[:, et, :])
            state[i] = {"x1": x1_sb, "x2": x2_sb, "x2r": x2r_sb}

        def emit_head(i):
            """Transposes + mm1 (PE-dense, needs only x1 + Wc)."""
            t = state[i]
            x1_sb = t["x1"]
            x1T_sb = pool_big.tile([P, DT, S], FP32, tag="x1T")  # [p, dt, s]
            for dt_ in range(DT):
                pst = ps_tr.tile([P, S], FP32, tag="ps_tr")
                for st_ in range(ST):
                    nc.tensor.matmul(
                        _mm(pst[:, st_ * P:(st_ + 1) * P]),
                        x1_sb[:, st_, dt_ * P:(dt_ + 1) * P],
                        identity[:], is_transpose=True,
                        start=(st_ == 0), stop=(st_ == ST - 1),
                    )
                if dt_ % 2 == 0:
                    nc.vector.tensor_copy(_mm(x1T_sb[:, dt_, :]), pst[:])
                else:
                    nc.scalar.copy(_mm(x1T_sb[:, dt_, :]), pst[:])

            # bias2 columns: gpsimd multiply, DVE free-axis reduce (the PE
            # pieces of the bias path run after mm1)
            b2c = pool_sm.tile([P, ST], FP32, tag="b2c")
            for st_ in range(ST):
                scr = pool_scr.tile([P, D], FP32, tag="scr")
                nc.vector.tensor_mul(scr[:], _fp(t["x2"][:, st_, :]), W2_rep[:])
                nc.vector.reduce_sum(b2c[:, st_:st_ + 1], scr[:], axis=AX)
            t["b2c"] = b2c

            x1wT_sb = pool_big.tile([P, DT, S], FP32, tag="x1wT")  # [p, et, s]
            for et in range(DT):
                ps = ps_mm.tile([P, S], FP32, tag="ps_mm")
                for dt_ in range(DT):
                    nc.tensor.matmul(
                        ps[:],
                        _mm(Wc_sb[:, dt_, et * P:(et + 1) * P]),
                        _mm(x1T_sb[:, dt_, :]),
                        start=(dt_ == 0), stop=(dt_ == DT - 1),
                    )
                nc.scalar.copy(_mm(x1wT_sb[:, et, :]), ps[:])
            t["x1T"] = x1T_sb
            t["x1wT"] = x1wT_sb

        def emit_bias_rows(i):
            t = state[i]
            ps_b1 = ps_sm.tile([1, S], FP32, tag="ps_row")
            for dt_ in range(DT):
                nc.tensor.matmul(
                    ps_b1[:], W1col[:, dt_:dt_ + 1], _mm(t["x1T"][:, dt_, :]),
                    start=(dt_ == 0), stop=(dt_ == DT - 1),
                )
            b1row_sb = pool_sm.tile([1, S], FP32, tag="b1row")
            nc.vector.tensor_copy(_mm(b1row_sb[:]), ps_b1[:])
            ps_b2 = ps_sm.tile([1, S], FP32, tag="ps_row")
            for c in range(ST):
                nc.tensor.matmul(
                    ps_b2[:, c * P:(c + 1) * P], t["b2c"][:, c:c + 1],
                    identity_f[:], is_transpose=True,
                    start=(c == 0), stop=(c == ST - 1),
                )
            b2row_sb = pool_sm.tile([1, S], FP32, tag="b2row")
            nc.vector.tensor_copy(_mm(b2row_sb[:]), ps_b2[:])
            t["b1row"] = b1row_sb
            t["b2row"] = b2row_sb

        def emit_mm2(i):
            t = state[i]
            A_sb = pool_a.tile([P, ST, S], FP32, tag="A")  # [p, st, t]
            for st_ in range(ST):
                ps = ps_mm.tile([P, S], FP32, tag="ps_mm")
                for et in range(DT):
                    nc.tensor.matmul(
                        ps[:],
                        _mm(t["x1wT"][:, et, st_ * P:(st_ + 1) * P]),
                        _mm(t["x2r"][:, et, :]),
                        start=(et == 0), stop=False,
                    )
                nc.tensor.matmul(ps[:], _mm(ones_col[:]), _mm(t["b2row"][:]),
                                 start=False, stop=True)
                nc.vector.tensor_copy(_mm(A_sb[:, st_, :]), ps[:])
            t["A"] = A_sb

        def emit_at(i):
            t = state[i]
            AT_sb = pool_a.tile([P, ST, S], FP32, tag="AT")  # [p, tt, s]
            for tt in range(ST):
                pst = ps_tr.tile([P, S], FP32, tag="ps_tr")
                for st_ in range(ST):
                    nc.tensor.matmul(
                        _mm(pst[:, st_ * P:(st_ + 1) * P]),
                        _mm(t["A"][:, st_, tt * P:(tt + 1) * P]),
                        identity[:], is_transpose=True,
                        start=(st_ == 0), stop=False,
                    )
                nc.tensor.matmul(pst[:], _mm(ones_col[:]), _mm(t["b1row"][:]),
                                 start=False, stop=True)
                if tt % 2 == 0:
                    nc.vector.tensor_copy(_mm(AT_sb[:, tt, :]), pst[:])
                else:
                    nc.scalar.copy(_mm(AT_sb[:, tt, :]), pst[:])
            t["AT"] = AT_sb

        def emit_softmax(buf):
            for j in range(ST):
                t_ap = buf[:, j, :]
                negmax = pool_sm.tile([P, 1], FP32, tag="negmax")
                nc.vector.reduce_max(negmax[:], t_ap, axis=AX, negate=True)
                den = pool_sm.tile([P, 1], FP32, tag="den")
                nc.scalar.activation(
                    _mm(t_ap), t_ap, EXP, bias=negmax[:], scale=1.0,
                    accum_out=den[:])
                rden = pool_sm.tile([P, 1], FP32, tag="rden")
                nc.vector.reciprocal(rden[:], den[:])
                nc.vector.tensor_scalar_mul(_mm(t_ap), t_ap, rden[:])

        def emit_f(i, b, out_dram, lhs_buf, rhs_buf, ftag):
            t = state[i]
            out_view = out_dram[b].rearrange("(st p) d -> p st d", p=P)
            for o in range(ST):
                fo = pool_f.tile([P, D], FP32, tag=ftag)
                for c in range(2):
                    ps = ps_f.tile([P, FC], FP32, tag="ps_f")
                    for k in range(ST):
                        nc.tensor.matmul(
                            ps[:],
                            _mm(t[lhs_buf][:, k, o * P:(o + 1) * P]),
                            _mm(t[rhs_buf][:, k, c * FC:(c + 1) * FC]),
                            start=(k == 0), stop=(k == ST - 1),
                        )
                    nc.scalar.copy(fo[:, c * FC:(c + 1) * FC], ps[:])
                nc.scalar.dma_start(out=out_view[:, o, :], in_=fo[:])

        # ---- software-pipelined emission across batches ----
        batches = [bb for _ in range(repeat) for bb in range(BPC)]
        n = len(batches)
        if PIPE:
            emit_loads(0, batches[0])
            emit_head(0)
            for i, b in enumerate(batches):
                emit_bias_rows(i)
                emit_mm2(i)
                if i + 1 < n:
                    emit_loads(i + 1, batches[i + 1])
                emit_at(i)
                emit_softmax(state[i]["AT"])
                if i + 1 < n:
                    emit_head(i + 1)
                emit_f(i, b, f1, "AT", "x2", "f1sb")
                emit_softmax(state[i]["A"])
                emit_f(i, b, f2, "A", "x1", "f2sb")
                del state[i]
        else:
            for i, b in enumerate(batches):
                emit_loads(i, b)
                emit_head(i)
                emit_bias_rows(i)
                emit_mm2(i)
                emit_at(i)
                emit_softmax(state[i]["AT"])
                emit_f(i, b, f1, "AT", "x2", "f1sb")
                emit_softmax(state[i]["A"])
                emit_f(i, b, f2, "A", "x1", "f2sb")
                del state[i]

    nc.finalize()
    return nc


_NC_CACHE = {}


def _get_nc(repeat=1):
    key = (MM_FAST, PIPE, os.environ.get("XATTN_BIGBUFS", "1"),
           os.environ.get("XATTN_PSTR", "2"), os.environ.get("XATTN_PSMM", "3"),
           os.environ.get("XATTN_PSF", "2"), os.environ.get("XATTN_FBUFS", "3"), os.environ.get("XATTN_XBUFS", "2"), repeat)
    if key not in _NC_CACHE:
        _NC_CACHE[key] = build_kernel(repeat=repeat)
    return _NC_CACHE[key]


def kernel(x1, x2, Wc, W1, W2):
    x1 = np.ascontiguousarray(x1, dtype=np.float32)
    x2 = np.ascontiguousarray(x2, dtype=np.float32)
    Wc = np.ascontiguousarray(Wc, dtype=np.float32)
    W1 = np.ascontiguousarray(W1, dtype=np.float32)
    W2 = np.ascontiguousarray(W2, dtype=np.float32)

    nc = _get_nc()
    in_maps = []
    for i in range(N_CORES):
        sl = slice(i * BPC, (i + 1) * BPC)
        in_maps.append(
            {"x1": x1[sl], "x2": x2[sl], "Wc": Wc, "W1": W1, "W2": W2}
        )
    res = run_bass_kernel_spmd(nc, in_maps, list(range(N_CORES)))
    f1 = np.concatenate([res.results[i]["f1"] for i in range(N_CORES)], axis=0)
    f2 = np.concatenate([res.results[i]["f2"] for i in range(N_CORES)], axis=0)
    return (f1, f2)

